# revision 10
# baseline (speedup 1.0000x reference)
"""Trainium2 Bass kernel: Bahdanau local-p attention (B=32, S=2048, H=1024).

Sharding: data-parallel over batch. Each of the 8 cores processes B/8 = 4
batches end-to-end (weights replicated); no collectives.

Per-core dataflow (all matmul-heavy work in fp16 with fp32 PSUM accumulation):
  1. inputs[b] is cast fp32->fp16 into DRAM (SWDGE cast DMA), then loaded
     transposed into SBUF via the xbar DMA-transpose path -> inT [h, s].
  2. WH^T tiles [h'=128, s=512] = W_a-tile^T @ inT  (PE, fp16).
  3. tanh(WH^T + U_a h_t) fused on ACT (per-partition bias), fp16 out.
  4. score = v_a^T tanh(...) via M=1 matmuls accumulated over h'-tiles.
  5. softmax (minus-max) + gaussian window + 1/sum on DVE/ACT rows.
  6. weights row replicated to 128 partitions via ones-matmul; context^T
     computed on DVE with fused multiply+reduce against inT.
  7. final tanh([ctx, h_t] @ W_att) via fp16 matmuls, fp32 out.
"""

import math
from contextlib import ExitStack

import numpy as np

B, S, H, SIZE = 32, 2048, 1024, 1024
N_CORES = 8
BPC = B // N_CORES
P = 128
NB = 512

_compiled = None


def _build(bpc=BPC, s=S, h=H, size=SIZE, debug=False):
    import concourse.bacc as bacc
    import concourse.mybir as mybir
    import concourse.tile as tile

    F32 = mybir.dt.float32
    F16 = mybir.dt.float16
    AF = mybir.ActivationFunctionType
    ALU = mybir.AluOpType
    AX = mybir.AxisListType

    KT = h // P          # k-tiles over H
    SQ = s // NB         # s blocks of 512
    KT2 = 2 * h // P     # k-tiles over 2H (final projection)
    NO = size // NB      # output blocks
    H2 = h // 2
    denom = 2.0 * ((s // 2) / 2.0) ** 2
    inv_sq_denom = 1.0 / math.sqrt(denom)

    nc = bacc.Bacc("TRN2", target_bir_lowering=False, debug=debug)

    x = nc.dram_tensor("inputs", [bpc, s, h], F32, kind="ExternalInput").ap()
    W_p = nc.dram_tensor("W_p", [h, h], F32, kind="ExternalInput").ap()
    v_p = nc.dram_tensor("v_p", [h, 1], F32, kind="ExternalInput").ap()
    W_a = nc.dram_tensor("W_a", [h, h], F32, kind="ExternalInput").ap()
    U_a = nc.dram_tensor("U_a", [h, h], F32, kind="ExternalInput").ap()
    v_a = nc.dram_tensor("v_a", [h, 1], F32, kind="ExternalInput").ap()
    W_att = nc.dram_tensor("W_att", [2 * h, size], F32, kind="ExternalInput").ap()
    out = nc.dram_tensor("out", [bpc, size], F32, kind="ExternalOutput").ap()

    with tile.TileContext(nc) as tc, ExitStack() as ctx:
        dp = ctx.enter_context(tc.tile_pool(name="dram", bufs=2, space="DRAM"))
        sb = ctx.enter_context(tc.tile_pool(name="sb", bufs=1))
        ps = ctx.enter_context(tc.tile_pool(name="ps", bufs=1, space="PSUM"))

        inT_tiles = [None] * bpc

        def emit_input_dma(b):
            # k-major fp16 staging in DRAM so each transpose source is a
            # fully contiguous 512KB block (fast xbar path).
            xf = dp.tile([KT, s, P], F16, name=f"xf16_{b}", tag="xf16")
            inT = sb.tile([P, KT, s], F16, name=f"inT_{b}", tag="big", bufs=2)
            for k in range(KT):
                nc.gpsimd.dma_start(xf[k], x[b, :, k * P:(k + 1) * P])
                nc.sync.dma_start(inT[:, k, :], xf[k], transpose=True)
            inT_tiles[b] = inT

        # ---- preamble weights first (PE executes preamble before batch 0),
        # then batch-0 inputs, then W_a ----
        ua_sb = sb.tile([P, KT, h], F16, name="ua_sb", tag="big", bufs=2)
        nc.gpsimd.dma_start(ua_sb[:], U_a.rearrange("(k p) n -> p k n", p=P))
        wp_sb = sb.tile([P, KT, h], F16, name="wp_sb", tag="wend")
        nc.gpsimd.dma_start(wp_sb[:], W_p.rearrange("(k p) n -> p k n", p=P))
        emit_input_dma(0)
        wa_sb = sb.tile([P, KT, h], F16, name="wa_sb", tag="wa")
        nc.gpsimd.dma_start(wa_sb[:], W_a.rearrange("(k p) n -> p k n", p=P))
        va_sb = sb.tile([P, KT], F16, name="va_sb", tag="va")
        nc.gpsimd.dma_start(va_sb[:], v_a.rearrange("(k p) o -> p (k o)", p=P))

        # ---- constants ----
        ident_io = sb.tile([bpc, bpc], F32, name="ident_io", tag="ident_io")
        nc.gpsimd.iota(ident_io[:], pattern=[[1, bpc]], base=0,
                       channel_multiplier=-1,
                       allow_small_or_imprecise_dtypes=True)
        ident = sb.tile([bpc, bpc], F32, name="ident", tag="ident")
        nc.vector.tensor_scalar(ident[:], ident_io[:], 0.0, None,
                                op0=ALU.is_equal)
        ones1 = sb.tile([1, P], F16, name="ones1", tag="ones1")
        nc.vector.memset(ones1[:], 1.0)
        pos_row = sb.tile([1, s], F32, name="pos_row", tag="pos")
        nc.gpsimd.iota(pos_row[:], pattern=[[1, s]], base=0,
                       channel_multiplier=0,
                       allow_small_or_imprecise_dtypes=True)

        # ---- h_t extraction + transposes ----
        htb = sb.tile([bpc, h], F32, name="htb", tag="htb")
        nc.scalar.dma_start(htb[:], x[:, s - 1, :])
        vp_rep = sb.tile([bpc, h], F32, name="vp_rep", tag="vp_rep")
        for i in range(bpc):
            nc.scalar.dma_start(vp_rep[i:i + 1, :], v_p.rearrange("n o -> o n"))

        htT = sb.tile([P, KT, bpc], F32, name="htT", tag="htT")
        htT16 = sb.tile([P, KT, bpc], F16, name="htT16", tag="htT16")
        combT = sb.tile([P, KT2, bpc], F16, name="combT", tag="combT")
        for k in range(KT):
            pt = ps.tile([P, bpc], F32, name=f"pt_{k}", tag="wh", bufs=4)
            nc.tensor.transpose(pt[:], htb[:, k * P:(k + 1) * P], ident[:])
            nc.scalar.activation(htT[:, k, :], pt[:], AF.Copy)
            nc.vector.tensor_copy(htT16[:, k, :], pt[:])
            nc.vector.tensor_copy(combT[:, KT + k, :], pt[:])

        # ---- WT = h_t @ U_a, then transpose -> wtT ----
        wt_row = sb.tile([bpc, h], F32, name="wt_row", tag="wt_row")
        for n2 in range(h // NB):
            pwt = ps.tile([bpc, NB], F32, name=f"pwt_{n2}", tag="sc", bufs=4)
            for k in range(KT):
                nc.tensor.matmul(pwt[:], htT16[:, k, :],
                                 ua_sb[:, k, n2 * NB:(n2 + 1) * NB],
                                 start=(k == 0), stop=(k == KT - 1))
            nc.scalar.activation(wt_row[:, n2 * NB:(n2 + 1) * NB], pwt[:], AF.Copy)
        wtT = sb.tile([P, KT, bpc], F32, name="wtT", tag="wtT")
        for k in range(KT):
            pt2 = ps.tile([P, bpc], F32, name=f"pt2_{k}", tag="wh", bufs=4)
            nc.tensor.transpose(pt2[:], wt_row[:, k * P:(k + 1) * P], ident[:])
            nc.scalar.activation(wtT[:, k, :], pt2[:], AF.Copy)

        # ---- p_t = sigmoid(tanh(h_t @ W_p) @ v_p) * s ----
        tanhP = sb.tile([bpc, h], F32, name="tanhP", tag="tanhP")
        for n2 in range(h // NB):
            pwp = ps.tile([bpc, NB], F32, name=f"pwp_{n2}", tag="sc", bufs=4)
            for k in range(KT):
                nc.tensor.matmul(pwp[:], htT16[:, k, :],
                                 wp_sb[:, k, n2 * NB:(n2 + 1) * NB],
                                 start=(k == 0), stop=(k == KT - 1))
            nc.scalar.activation(tanhP[:, n2 * NB:(n2 + 1) * NB], pwp[:], AF.Tanh)
        scrP = sb.tile([bpc, h], F32, name="scrP", tag="scrP")
        z2t = sb.tile([bpc, 1], F32, name="z2t", tag="z2t")
        nc.vector.scalar_tensor_tensor(
            scrP[:], tanhP[:], 1.0, vp_rep[:],
            op0=ALU.mult, op1=ALU.mult, accum_out=z2t[:])
        pz = ps.tile([1, bpc], F32, name="pz", tag="sc", bufs=4)
        nc.tensor.transpose(pz[:], z2t[:], ident[:])
        sg_row = sb.tile([1, bpc], F32, name="sg_row", tag="sg_row")
        nc.scalar.activation(sg_row[:], pz[:], AF.Sigmoid)
        p_row = sb.tile([1, bpc], F32, name="p_row", tag="p_row")
        nc.vector.tensor_scalar_mul(p_row[:], sg_row[:], float(s))

        # ---- watt (emitted during batch 1 prefetch; declared here) ----
        watt_holder = [None]

        def emit_watt_dma():
            watt_sb = sb.tile([P, KT2, size], F16, name="watt_sb", tag="wend")
            nc.gpsimd.dma_start(watt_sb[:],
                                W_att.rearrange("(k p) n -> p k n", p=P))
            watt_holder[0] = watt_sb

        # ---- main batch loop ----
        for b in range(bpc):
            if b + 1 < bpc:
                emit_input_dma(b + 1)
            if b == 1 or (bpc == 1):
                emit_watt_dma()
            inT = inT_tiles[b]

            sc_ps = [ps.tile([1, NB], F32, name=f"sc_{b}_{q}", tag="sc", bufs=4)
                     for q in range(SQ)]
            for hp in range(KT):
                wh_ps = [ps.tile([P, NB], F32, name=f"wh_{b}_{hp}_{q}",
                                 tag="wh", bufs=4) for q in range(SQ)]
                for k in range(KT):
                    lhsT = wa_sb[:, k, hp * P:(hp + 1) * P]
                    for q in range(SQ):
                        nc.tensor.matmul(
                            wh_ps[q][:], lhsT, inT[:, k, q * NB:(q + 1) * NB],
                            start=(k == 0), stop=(k == KT - 1),
                            skip_group_check=True)
                for q in range(SQ):
                    th = sb.tile([P, NB], F16, name=f"th_{b}_{hp}_{q}",
                                 tag="tanh", bufs=6)
                    nc.scalar.activation(th[:], wh_ps[q][:], AF.Tanh,
                                         bias=wtT[:, hp, b:b + 1])
                    nc.tensor.matmul(sc_ps[q][:], va_sb[:, hp:hp + 1], th[:],
                                     start=(hp == 0), stop=(hp == KT - 1),
                                     skip_group_check=True)

            # ---- softmax * gaussian (rows on partition 0) ----
            score = sb.tile([1, s], F32, name=f"score_{b}", tag="score")
            for q in range(SQ):
                nc.vector.tensor_copy(score[0:1, q * NB:(q + 1) * NB],
                                      sc_ps[q][:])
            nmx = sb.tile([1, 1], F32, name=f"nmx_{b}", tag="nmx", bufs=2)
            nc.vector.tensor_reduce(nmx[:], score[:], axis=AX.X, op=ALU.max,
                                    negate=True)
            e1 = sb.tile([1, s], F16, name=f"e1_{b}", tag="e1")
            nc.scalar.activation(e1[:], score[:], AF.Exp, bias=nmx[0:1, 0:1])
            se = sb.tile([1, 1], F32, name=f"se_{b}", tag="se", bufs=2)
            nc.vector.tensor_reduce(se[:], e1[:], axis=AX.X, op=ALU.add)
            rr = sb.tile([1, 1], F32, name=f"rr_{b}", tag="rr", bufs=2)
            nc.vector.reciprocal(rr[:], se[:])
            dr = sb.tile([1, s], F16, name=f"dr_{b}", tag="gA")
            nc.vector.tensor_scalar(dr[:], pos_row[:], p_row[0:1, b:b + 1],
                                    inv_sq_denom, op0=ALU.subtract,
                                    op1=ALU.mult)
            d2 = sb.tile([1, s], F16, name=f"d2_{b}", tag="gB")
            nc.vector.tensor_mul(d2[:], dr[:], dr[:])
            gr = sb.tile([1, s], F16, name=f"gr_{b}", tag="gA")
            nc.scalar.activation(gr[:], d2[:], AF.Exp, scale=-1.0)
            wu = sb.tile([1, s], F16, name=f"wu_{b}", tag="gB")
            nc.vector.scalar_tensor_tensor(wu[:], e1[:], rr[0:1, 0:1], gr[:],
                                           op0=ALU.mult, op1=ALU.mult)

            # ---- replicate weights row across partitions ----
            wrep = sb.tile([P, s], F16, name=f"wrep_{b}", tag="wrep", bufs=2)
            for q in range(SQ):
                pwr = ps.tile([P, NB], F32, name=f"pwr_{b}_{q}", tag="sc",
                              bufs=4)
                nc.tensor.matmul(pwr[:], ones1[0:1, :],
                                 wu[0:1, q * NB:(q + 1) * NB],
                                 start=True, stop=True, skip_group_check=True)
                nc.scalar.activation(wrep[:, q * NB:(q + 1) * NB], pwr[:],
                                     AF.Copy)

            # ---- context^T via fused multiply+reduce on DVE ----
            ctxa = sb.tile([P, KT], F32, name=f"ctxa_{b}", tag="ctxa", bufs=2)
            for k in range(KT):
                cs = sb.tile([P, s], F16, name=f"cs_{b}_{k}", tag="ctxs",
                             bufs=2)
                nc.vector.scalar_tensor_tensor(
                    cs[:], inT[:, k, :], 1.0, wrep[:],
                    op0=ALU.mult, op1=ALU.mult,
                    accum_out=ctxa[:, k:k + 1])
            nc.vector.tensor_copy(combT[:, 0:KT, b], ctxa[:])

        # ---- final projection: tanh([ctx, h_t] @ W_att) ----
        watt_sb = watt_holder[0]
        outsb = sb.tile([bpc, size], F32, name="outsb", tag="outsb")
        for n2 in range(NO):
            pf = ps.tile([bpc, NB], F32, name=f"pf_{n2}", tag="sc", bufs=4)
            for kk in range(KT2):
                nc.tensor.matmul(pf[:], combT[:, kk, :],
                                 watt_sb[:, kk, n2 * NB:(n2 + 1) * NB],
                                 start=(kk == 0), stop=(kk == KT2 - 1),
                                 skip_group_check=True)
            nc.scalar.activation(outsb[:, n2 * NB:(n2 + 1) * NB], pf[:],
                                 AF.Tanh)
        nc.scalar.dma_start(out[:], outsb[:])

    nc.compile()
    return nc


def kernel(**inputs):
    global _compiled
    from concourse import bass_utils

    if _compiled is None:
        _compiled = _build()

    x = np.ascontiguousarray(np.asarray(inputs["inputs"], dtype=np.float32))
    weights = {
        k: np.ascontiguousarray(np.asarray(inputs[k], dtype=np.float32))
        for k in ("W_p", "v_p", "W_a", "U_a", "v_a", "W_att")
    }
    in_maps = [
        {"inputs": x[i * BPC:(i + 1) * BPC], **weights} for i in range(N_CORES)
    ]
    res = bass_utils.run_bass_kernel_spmd(_compiled, in_maps,
                                          list(range(N_CORES)))
    return np.concatenate([res.results[i]["out"] for i in range(N_CORES)],
                          axis=0).astype(np.float32)


# revision 25
# speedup vs baseline: 1.2085x; 1.2085x over previous
"""Trainium2 Bass kernel: Bahdanau local-p attention (B=32, S=2048, H=1024).

Sharding: data-parallel over batch. Each of the 8 cores processes B/8 = 4
batches end-to-end (weights replicated); no collectives.

Per-core dataflow (all matmul-heavy work in fp16 with fp32 PSUM accumulation):
  1. inputs[b] is cast fp32->fp16 into DRAM (SWDGE cast DMA), then loaded
     transposed into SBUF via the xbar DMA-transpose path -> inT [h, s].
  2. WH^T tiles [h'=128, s=512] = W_a-tile^T @ inT  (PE, fp16).
  3. tanh(WH^T + U_a h_t) fused on ACT (per-partition bias), fp16 out.
  4. score = v_a^T tanh(...) via M=1 matmuls accumulated over h'-tiles.
  5. softmax (minus-max) + gaussian window + 1/sum on DVE/ACT rows.
  6. weights row replicated to 128 partitions via ones-matmul; context^T
     computed on DVE with fused multiply+reduce against inT.
  7. final tanh([ctx, h_t] @ W_att) via fp16 matmuls, fp32 out.
"""

import math
from contextlib import ExitStack

import numpy as np

B, S, H, SIZE = 32, 2048, 1024, 1024
N_CORES = 8
BPC = B // N_CORES
P = 128
NB = 512

_compiled = None


def _build(bpc=BPC, s=S, h=H, size=SIZE, debug=False):
    import concourse.bacc as bacc
    import concourse.mybir as mybir
    import concourse.tile as tile

    F32 = mybir.dt.float32
    F16 = mybir.dt.float16
    AF = mybir.ActivationFunctionType
    ALU = mybir.AluOpType
    AX = mybir.AxisListType

    KT = h // P          # k-tiles over H
    SQ = s // NB         # s blocks of 512
    KT2 = 2 * h // P     # k-tiles over 2H (final projection)
    NO = size // NB      # output blocks
    H2 = h // 2
    denom = 2.0 * ((s // 2) / 2.0) ** 2
    inv_sq_denom = 1.0 / math.sqrt(denom)

    nc = bacc.Bacc("TRN2", target_bir_lowering=False, debug=debug)

    x = nc.dram_tensor("inputs", [bpc, s, h], F32, kind="ExternalInput").ap()
    W_p = nc.dram_tensor("W_p", [h, h], F32, kind="ExternalInput").ap()
    v_p = nc.dram_tensor("v_p", [h, 1], F32, kind="ExternalInput").ap()
    W_a = nc.dram_tensor("W_a", [h, h], F32, kind="ExternalInput").ap()
    U_a = nc.dram_tensor("U_a", [h, h], F32, kind="ExternalInput").ap()
    v_a = nc.dram_tensor("v_a", [h, 1], F32, kind="ExternalInput").ap()
    W_att = nc.dram_tensor("W_att", [2 * h, size], F32, kind="ExternalInput").ap()
    out = nc.dram_tensor("out", [bpc, size], F32, kind="ExternalOutput").ap()

    with tile.TileContext(nc) as tc, ExitStack() as ctx:
        dp = ctx.enter_context(tc.tile_pool(name="dram", bufs=2, space="DRAM"))
        sb = ctx.enter_context(tc.tile_pool(name="sb", bufs=1))
        ps = ctx.enter_context(tc.tile_pool(name="ps", bufs=1, space="PSUM"))

        inT_tiles = [None] * bpc

        def emit_input_dma(b):
            # single fully-contiguous fp32->fp16 cast DMA (SWDGE), then xbar
            # transposes of the k-th 128-column slice.
            xf = dp.tile([s, h], F16, name=f"xf16_{b}", tag="xf16")
            nc.gpsimd.dma_start(xf[:], x[b])
            inT = sb.tile([P, KT, s], F16, name=f"inT_{b}", tag="big", bufs=2)
            for k in range(KT):
                nc.sync.dma_start(inT[:, k, :], xf[:, k * P:(k + 1) * P],
                                  transpose=True)
            inT_tiles[b] = inT

        # ---- SWDGE queue order: b0 cast, U_a, W_a, v_a, b1 cast, W_p,
        # then b2/b3/W_att from the batch loop. HWDGE carries ONLY the xbar
        # transposes (mixing copy-DMAs with transposes serializes globally),
        # plus a few tiny DMAs emitted before the first transpose / at exit.
        def load_weight_f16(name, dram_ap, kt, n, tag, bufs):
            w16 = sb.tile([P, kt, n], F16, name=name, tag=tag, bufs=bufs)
            nc.gpsimd.dma_start(w16[:],
                                dram_ap.rearrange("(k p) n -> p k n", p=P))
            return w16

        htb = sb.tile([bpc, h], F32, name="htb", tag="htb")
        nc.scalar.dma_start(htb[:], x[:, s - 1, :])
        vp_rep = sb.tile([bpc, h], F32, name="vp_rep", tag="vp_rep")
        for i in range(bpc):
            nc.scalar.dma_start(vp_rep[i:i + 1, :], v_p.rearrange("n o -> o n"))
        emit_input_dma(0)
        ua_sb = load_weight_f16("ua_sb", U_a, KT, h, "ua", 1)
        wa_sb = load_weight_f16("wa_sb", W_a, KT, h, "wa", 1)
        va_sb = sb.tile([P, KT], F16, name="va_sb", tag="va")
        nc.gpsimd.dma_start(va_sb[:], v_a.rearrange("(k p) o -> p (k o)", p=P))
        emit_input_dma(1)
        wp_sb = load_weight_f16("wp_sb", W_p, KT, h, "wend", 1)

        # ---- constants ----
        ident_io = sb.tile([bpc, bpc], F32, name="ident_io", tag="ident_io")
        nc.gpsimd.iota(ident_io[:], pattern=[[1, bpc]], base=0,
                       channel_multiplier=-1,
                       allow_small_or_imprecise_dtypes=True)
        ident = sb.tile([bpc, bpc], F32, name="ident", tag="ident")
        nc.vector.tensor_scalar(ident[:], ident_io[:], 0.0, None,
                                op0=ALU.is_equal)
        ones1 = sb.tile([1, P], F16, name="ones1", tag="ones1")
        nc.vector.memset(ones1[:], 1.0)
        pos_row = sb.tile([1, s], F16, name="pos_row", tag="pos")
        nc.gpsimd.iota(pos_row[:], pattern=[[1, s]], base=0,
                       channel_multiplier=0,
                       allow_small_or_imprecise_dtypes=True)

        # ---- h_t extraction + transposes ----


        htT = sb.tile([P, KT, bpc], F32, name="htT", tag="htT")
        htT16 = sb.tile([P, KT, bpc], F16, name="htT16", tag="htT16")
        combT = sb.tile([P, KT2, bpc], F16, name="combT", tag="combT")
        for k in range(KT):
            pt = ps.tile([P, bpc], F32, name=f"pt_{k}", tag="wh", bufs=4)
            nc.tensor.transpose(pt[:], htb[:, k * P:(k + 1) * P], ident[:])
            nc.scalar.activation(htT[:, k, :], pt[:], AF.Copy)
            nc.vector.tensor_copy(htT16[:, k, :], pt[:])
            nc.vector.tensor_copy(combT[:, KT + k, :], pt[:])

        # ---- WT = h_t @ U_a, then transpose -> wtT ----
        wt_row = sb.tile([bpc, h], F32, name="wt_row", tag="wt_row")
        for n2 in range(h // NB):
            pwt = ps.tile([bpc, NB], F32, name=f"pwt_{n2}", tag="sc", bufs=4)
            for k in range(KT):
                nc.tensor.matmul(pwt[:], htT16[:, k, :],
                                 ua_sb[:, k, n2 * NB:(n2 + 1) * NB],
                                 start=(k == 0), stop=(k == KT - 1))
            nc.scalar.activation(wt_row[:, n2 * NB:(n2 + 1) * NB], pwt[:], AF.Copy)
        wtT = sb.tile([P, KT, bpc], F32, name="wtT", tag="wtT")
        for k in range(KT):
            pt2 = ps.tile([P, bpc], F32, name=f"pt2_{k}", tag="wh", bufs=4)
            nc.tensor.transpose(pt2[:], wt_row[:, k * P:(k + 1) * P], ident[:])
            nc.scalar.activation(wtT[:, k, :], pt2[:], AF.Copy)

        # ---- p_t = sigmoid(tanh(h_t @ W_p) @ v_p) * s ----
        tanhP = sb.tile([bpc, h], F32, name="tanhP", tag="tanhP")
        for n2 in range(h // NB):
            pwp = ps.tile([bpc, NB], F32, name=f"pwp_{n2}", tag="sc", bufs=4)
            for k in range(KT):
                nc.tensor.matmul(pwp[:], htT16[:, k, :],
                                 wp_sb[:, k, n2 * NB:(n2 + 1) * NB],
                                 start=(k == 0), stop=(k == KT - 1))
            nc.scalar.activation(tanhP[:, n2 * NB:(n2 + 1) * NB], pwp[:], AF.Tanh)
        z2t = sb.tile([bpc, 1], F32, name="z2t", tag="z2t")
        nc.vector.scalar_tensor_tensor(
            tanhP[:], tanhP[:], 1.0, vp_rep[:],
            op0=ALU.mult, op1=ALU.mult, accum_out=z2t[:])
        pz = ps.tile([1, bpc], F32, name="pz", tag="sc", bufs=4)
        nc.tensor.transpose(pz[:], z2t[:], ident[:])
        sg_row = sb.tile([1, bpc], F32, name="sg_row", tag="sg_row")
        nc.scalar.activation(sg_row[:], pz[:], AF.Sigmoid)
        p_row = sb.tile([1, bpc], F32, name="p_row", tag="p_row")
        nc.vector.tensor_scalar_mul(p_row[:], sg_row[:], float(s))

        # ---- watt (emitted during batch 1 prefetch; declared here) ----
        watt_holder = [None]

        def emit_watt_dma():
            watt_holder[0] = load_weight_f16("watt_sb", W_att, KT2, size,
                                             "wend", 1)

        # ---- main batch loop ----
        for b in range(bpc):
            if b + 2 < bpc:
                emit_input_dma(b + 2)
            if b == 1 or bpc <= 2:
                emit_watt_dma()
            inT = inT_tiles[b]

            sc_ps = [ps.tile([1, NB], F32, name=f"sc_{b}_{q}", tag="sc", bufs=4)
                     for q in range(SQ)]

            def emit_va_mms(hp, tanh_tiles):
                for q in range(SQ):
                    nc.tensor.matmul(sc_ps[q][:], va_sb[:, hp:hp + 1],
                                     tanh_tiles[q][:],
                                     start=(hp == 0), stop=(hp == KT - 1),
                                     skip_group_check=True)

            # v_a matmuls run one hp-group behind the main matmuls so the PE
            # never waits on ACT's tanh.
            pend = None
            for hp in range(KT):
                wh_ps = [ps.tile([P, NB], F32, name=f"wh_{b}_{hp}_{q}",
                                 tag="wh", bufs=4) for q in range(SQ)]
                for k in range(KT):
                    lhsT = wa_sb[:, k, hp * P:(hp + 1) * P]
                    for q in range(SQ):
                        nc.tensor.matmul(
                            wh_ps[q][:], lhsT, inT[:, k, q * NB:(q + 1) * NB],
                            start=(k == 0), stop=(k == KT - 1),
                            skip_group_check=True)
                if pend is not None:
                    emit_va_mms(hp - 1, pend)
                ths = []
                for q in range(SQ):
                    th = sb.tile([P, NB], F16, name=f"th_{b}_{hp}_{q}",
                                 tag="tanh", bufs=8)
                    nc.scalar.activation(th[:], wh_ps[q][:], AF.Tanh,
                                         bias=wtT[:, hp, b:b + 1])
                    ths.append(th)
                pend = ths
            emit_va_mms(KT - 1, pend)

            # ---- softmax * gaussian (rows on partition 0) ----
            score = sb.tile([1, s], F32, name=f"score_{b}", tag="score")
            for q in range(SQ):
                nc.vector.tensor_copy(score[0:1, q * NB:(q + 1) * NB],
                                      sc_ps[q][:])
            nmx = sb.tile([1, 1], F32, name=f"nmx_{b}", tag="nmx", bufs=2)
            nc.vector.tensor_reduce(nmx[:], score[:], axis=AX.X, op=ALU.max,
                                    negate=True)
            e1 = sb.tile([1, s], F16, name=f"e1_{b}", tag="e1")
            nc.scalar.activation(e1[:], score[:], AF.Exp, bias=nmx[0:1, 0:1])
            se = sb.tile([1, 1], F32, name=f"se_{b}", tag="se", bufs=2)
            nc.vector.tensor_reduce(se[:], e1[:], axis=AX.X, op=ALU.add)
            rr = sb.tile([1, 1], F32, name=f"rr_{b}", tag="rr", bufs=2)
            nc.vector.reciprocal(rr[:], se[:])
            dr = sb.tile([1, s], F16, name=f"dr_{b}", tag="gA")
            nc.vector.tensor_scalar(dr[:], pos_row[:], p_row[0:1, b:b + 1],
                                    inv_sq_denom, op0=ALU.subtract,
                                    op1=ALU.mult)
            d2 = sb.tile([1, s], F16, name=f"d2_{b}", tag="gB")
            nc.vector.tensor_mul(d2[:], dr[:], dr[:])
            gr = sb.tile([1, s], F16, name=f"gr_{b}", tag="gA")
            nc.scalar.activation(gr[:], d2[:], AF.Exp, scale=-1.0)
            wu = sb.tile([1, s], F16, name=f"wu_{b}", tag="gB")
            nc.vector.scalar_tensor_tensor(wu[:], e1[:], rr[0:1, 0:1], gr[:],
                                           op0=ALU.mult, op1=ALU.mult)

            # ---- replicate weights row across partitions ----
            wrep = sb.tile([P, s], F16, name=f"wrep_{b}", tag="wrep", bufs=2)
            for q in range(SQ):
                pwr = ps.tile([P, NB], F32, name=f"pwr_{b}_{q}", tag="sc",
                              bufs=4)
                nc.tensor.matmul(pwr[:], ones1[0:1, :],
                                 wu[0:1, q * NB:(q + 1) * NB],
                                 start=True, stop=True, skip_group_check=True)
                nc.scalar.activation(wrep[:, q * NB:(q + 1) * NB], pwr[:],
                                     AF.Copy)

            # ---- context^T via fused multiply+reduce on DVE ----
            ctxa = sb.tile([P, KT], F32, name=f"ctxa_{b}", tag="ctxa", bufs=2)
            for k in range(KT):
                eng = nc.vector
                eng.scalar_tensor_tensor(
                    inT[:, k, :], inT[:, k, :], 1.0, wrep[:],
                    op0=ALU.mult, op1=ALU.mult,
                    accum_out=ctxa[:, k:k + 1])
            nc.vector.tensor_copy(combT[:, 0:KT, b], ctxa[:])

        # ---- final projection: tanh([ctx, h_t] @ W_att) ----
        watt_sb = watt_holder[0]
        outsb = sb.tile([bpc, size], F32, name="outsb", tag="outsb")
        for n2 in range(NO):
            pf = ps.tile([bpc, NB], F32, name=f"pf_{n2}", tag="sc", bufs=4)
            for kk in range(KT2):
                nc.tensor.matmul(pf[:], combT[:, kk, :],
                                 watt_sb[:, kk, n2 * NB:(n2 + 1) * NB],
                                 start=(kk == 0), stop=(kk == KT2 - 1),
                                 skip_group_check=True)
            nc.scalar.activation(outsb[:, n2 * NB:(n2 + 1) * NB], pf[:],
                                 AF.Tanh)
        nc.scalar.dma_start(out[:], outsb[:])

    nc.compile()
    return nc


def kernel(**inputs):
    global _compiled
    from concourse import bass_utils

    if _compiled is None:
        _compiled = _build()

    x = np.ascontiguousarray(np.asarray(inputs["inputs"], dtype=np.float32))
    weights = {
        k: np.ascontiguousarray(np.asarray(inputs[k], dtype=np.float32))
        for k in ("W_p", "v_p", "W_a", "U_a", "v_a", "W_att")
    }
    in_maps = [
        {"inputs": x[i * BPC:(i + 1) * BPC], **weights} for i in range(N_CORES)
    ]
    res = bass_utils.run_bass_kernel_spmd(_compiled, in_maps,
                                          list(range(N_CORES)))
    return np.concatenate([res.results[i]["out"] for i in range(N_CORES)],
                          axis=0).astype(np.float32)
